# revision 6
# baseline (speedup 1.0000x reference)
"""Trainium2 Bass kernel for Gumbel 2:4-masked Linear (tensor-parallel over out_features).

Forward value (matches reference): mask = PATTERNS[argmax(cw + g, axis=-1)],
out = x @ (W * mask).T + b.  With constant choice_weights the argmax is
shift-invariant, so the mask is PATTERNS[argmax(g)].

v2 design (cost-model driven, walrus-ISA-legal):
  - bf16 GEMM (x, W, masked W): same PE rate as f32r in the cost model but
    half the DMA traffic and 2x DVE rate on mask math. Gumbel stays f32
    (bf16 would flip ~1e-3 of the argmaxes -> ~3e-2 error).
  - Mask build per (kc, ot) tile [128 o, 256 blocks]: one max-reduce and one
    batched is_ge (all 6 planes, broadcast max) on DVE (Pool's GPSIMD has no
    max/compare opcodes); 4 batched bf16 adds form the mask columns and one
    flat bf16 mul masks W -- those run on Pool for POOL_ITERS to offload DVE.
  - PE transposes masked-W subtiles via identity matmuls into PSUM; Act
    (otherwise idle) copies PSUM -> resident wmT tiles.
  - GEMM: 16 token strips x 2 PSUM chains of 32 accumulating matmuls over
    resident wmT; bias added during the DVE PSUM->SBUF copy (broadcast add);
    DMA out bf16, host upcasts to f32.
  - Host pre-packs xT/W (plane-split 2:4 layout, bf16) so every DMA moves
    >=2KB contiguous runs (full 360 GB/s in the model); per-strip xT
    descriptors are 8-16 KB.
  - xt strip loads interleaved into phase 1 at chunk boundaries so the GEMM
    starts as soon as wmt[0] lands; 7 PSUM acc slots + 1 transpose slot.
"""

import numpy as np

N_CORES = 8
T = 4096            # tokens
K = 4096            # in_features
O_FULL = 4096
O = O_FULL // N_CORES        # 512 out rows per core
B = K // 4                   # 1024 blocks per out row
GUM_COLS = B * 6             # 6144
N_KC = 4                     # k chunks
KC_B = B // N_KC             # 256 blocks per chunk
N_OT = O // 128              # 4 o-tiles
N_STRIP = 16                 # token strips
TS = T // N_STRIP            # 256 tokens per strip

import os as _os

# iters whose reduce+compare run on Pool (rest on DVE); tuned for balance
_POOL_SETS = {
    "16": list(range(16)),
    "14": [i for i in range(16) if i not in (0, 3)],
    "12": [1, 2, 4, 5, 6, 8, 9, 11, 12, 13, 14, 15],
    "10": [1, 2, 5, 6, 8, 9, 11, 12, 14, 15],
    "8": [1, 3, 5, 7, 8, 10, 12, 14],
    "10c": [0, 1, 2, 5, 6, 8, 9, 11, 12, 14],
    "10d": [0, 1, 2, 3, 5, 6, 8, 9, 12, 14],
    "11b": [0, 1, 2, 3, 5, 6, 8, 9, 11, 12, 14],
    "12b": [0, 1, 2, 3, 4, 5, 6, 8, 9, 11, 12, 14],
    "10e": [0, 1, 2, 4, 6, 8, 9, 11, 12, 14],
    "10f": [0, 1, 2, 5, 6, 8, 10, 11, 13, 14],
    "9c": [0, 1, 2, 5, 6, 8, 9, 12, 14],
    "11c": [0, 1, 2, 5, 6, 8, 9, 11, 12, 14, 15],
}
POOL_ITERS = frozenset(_POOL_SETS[_os.environ.get("KV2_POOL", "10c")])
# strips using bias-matmul zeroing + o-sliced chunk-0 + Act-copy out
N_EARLY = int(_os.environ.get("KV2_EARLY", "0"))
# xt (strip, half) prefetches at each chunk boundary
_XT_PLANS = {
    "A": [[(0, 0), (1, 0)], [(2, 0), (0, 1)], [(1, 1), (3, 0), (2, 1)],
          [(3, 1)]],
    "B": [[(0, 0)], [(1, 0)], [(0, 1), (2, 0)], [(1, 1), (3, 0)]],
    "C": [[(0, 0)], [(1, 0), (2, 0)], [(0, 1), (1, 1)], [(2, 1), (3, 0)]],
    "D": [[(0, 0)], [(1, 0)], [(2, 0)], [(0, 1), (3, 0)]],
    "E": [[(0, 0)], [(1, 0), (2, 0)], [(3, 0), (0, 1), (4, 0)],
          [(1, 1), (5, 0)]],
    "F": [[(0, 0)], [(1, 0), (2, 0)], [(3, 0), (4, 0), (0, 1)],
          [(5, 0), (1, 1), (6, 0), (2, 1)]],
    "G": [[(0, 0), (1, 0)], [(2, 0), (3, 0)], [(4, 0), (0, 1), (5, 0)],
          [(1, 1), (6, 0), (2, 1)]],
}
XT_PLAN = _XT_PLANS[_os.environ.get("KV2_XT", "C")]

_prog_cache = {}


def _build_program(mode):
    """mode: 'const' (choice folded away) or 'full' (adds choice tensor)."""
    import concourse.bacc as bacc
    import concourse.bass as bass
    import concourse.mybir as mybir
    import concourse.tile as tile
    from concourse.masks import make_identity

    f32 = mybir.dt.float32
    bf16 = mybir.dt.bfloat16
    Alu = mybir.AluOpType

    nc = bacc.Bacc(trn_type="TRN2")
    # xt packed: [strip, kbp, (kc, s, kbt), t] -> [16*128, 32*256] bf16
    xt_d = nc.declare_dram_parameter("xt", [N_STRIP * 128, 32 * TS], bf16,
                                     isOutput=False)
    # w packed: [(kc, ot), po, (s, kb)] -> [16*128, 1024] bf16
    w_d = nc.declare_dram_parameter("w", [16 * 128, 1024], bf16, isOutput=False)
    b_d = nc.declare_dram_parameter("b", [1, O], f32, isOutput=False)
    g_d = nc.declare_dram_parameter("g", [O, GUM_COLS], f32, isOutput=False)
    if mode == "full":
        cw_d = nc.declare_dram_parameter("cw", [O, GUM_COLS], f32, isOutput=False)
    out_d = nc.declare_dram_parameter("out", [T, O], bf16, isOutput=True)

    with tile.TileContext(nc) as tc:
        with (
            tc.tile_pool(name="singles", bufs=1) as singles,
            tc.tile_pool(name="wmt", bufs=N_KC) as wmt_pool,
            tc.tile_pool(name="gum", bufs=4) as gum_pool,
            tc.tile_pool(name="wtile", bufs=4) as w_pool,
            tc.tile_pool(name="msk", bufs=3) as msk,
            tc.tile_pool(name="xth", bufs=8) as xt_pool,
            tc.tile_pool(name="outs", bufs=3) as out_pool,
            tc.tile_pool(name="ps_xpose", bufs=int(_os.environ.get("KV2_XPB", "1")),
                         space="PSUM") as ps_xpose,
            tc.tile_pool(name="ps_gemm", bufs=int(_os.environ.get("KV2_ACCB", "7")),
                         space="PSUM") as ps_gemm,
        ):
            ident_f32 = singles.tile([128, 128], f32)
            make_identity(nc, ident_f32)
            ident = singles.tile([128, 128], bf16, name="ident_bf")
            nc.vector.tensor_copy(ident, ident_f32)
            bias_s = singles.tile([128, O], f32, name="bias_s")
            nc.gpsimd.dma_start(
                out=bias_s,
                in_=bass.AP(tensor=b_d, offset=0, ap=[[0, 128], [1, O]]),
            )
            bias_bf = singles.tile([1, O], bf16, name="bias_bf")
            nc.vector.tensor_copy(bias_bf, bias_s[0:1, :])
            ones_t = singles.tile([1, 128], bf16, name="ones")
            nc.vector.memset(ones_t, 1.0)

            # resident transposed masked weight, one tile per k chunk:
            # wmt[kc][kbp, j=(s,kbt), o]
            wmt = [
                wmt_pool.tile([128, 8, O], bf16, name=f"wmt{i}", tag=f"wmt{i}",
                              bufs=1)
                for i in range(N_KC)
            ]

            xt_tiles = {}  # (strip, half) -> tile

            def issue_xt(s, h):
                if (s, h) in xt_tiles:
                    return
                t_ = xt_pool.tile([128, 16, TS], bf16, tag="xth")
                nc.sync.dma_start(
                    out=t_,
                    in_=xt_d[s * 128:(s + 1) * 128,
                             h * 16 * TS:(h + 1) * 16 * TS],
                )
                xt_tiles[(s, h)] = t_

            # ---------------- phase 1: masks + masked W^T -----------------
            for kc in range(N_KC):
                g_tiles = [None] * N_OT
                w_tiles = [None] * N_OT

                def _load_g(ot, kc=kc, g_tiles=g_tiles):
                    rows = slice(ot * 128, (ot + 1) * 128)
                    t_ = gum_pool.tile([128, KC_B * 6], f32, tag="gum",
                                       name=f"g_{kc}_{ot}")
                    nc.sync.dma_start(
                        out=t_, in_=g_d[rows, kc * KC_B * 6:(kc + 1) * KC_B * 6]
                    )
                    g_tiles[ot] = t_

                def _load_w(ot, kc=kc, w_tiles=w_tiles):
                    t_ = w_pool.tile([128, 1024], bf16, tag="w",
                                     name=f"w_{kc}_{ot}")
                    nc.sync.dma_start(
                        out=t_, in_=w_d[(kc * N_OT + ot) * 128:
                                        (kc * N_OT + ot + 1) * 128, :]
                    )
                    w_tiles[ot] = t_

                if kc == 0:
                    # cold start: g00 first (mask pipe), then all w (muls),
                    # then remaining g
                    _load_g(0)
                    for ot in range(N_OT):
                        _load_w(ot)
                    for ot in range(1, N_OT):
                        _load_g(ot)
                else:
                    for ot in range(N_OT):
                        _load_g(ot)
                    for ot in range(N_OT):
                        _load_w(ot)
                for ot in range(N_OT):
                    it = kc * N_OT + ot
                    # Pool is add/sub/mult-only on TRN2; reduce+compare are
                    # DVE-only. Pool takes adds+mul for POOL_ITERS.
                    eng = nc.gpsimd if it in POOL_ITERS else nc.vector
                    rows = slice(ot * 128, (ot + 1) * 128)
                    g_t = g_tiles[ot]
                    if mode == "full":
                        cw_t = gum_pool.tile([128, KC_B * 6], f32, tag="cw")
                        nc.sync.dma_start(
                            out=cw_t,
                            in_=cw_d[rows, kc * KC_B * 6:(kc + 1) * KC_B * 6],
                        )
                        nc.vector.tensor_add(g_t, g_t, cw_t)
                    g3 = g_t.rearrange("p (b s) -> p b s", s=6)

                    m = msk.tile([128, KC_B], f32, tag="m")
                    nc.vector.tensor_reduce(m, g3, axis=mybir.AxisListType.X,
                                            op=Alu.max)

                    # one-hot planes e[s][kb] (multi-hot only on exact ties)
                    e_t = msk.tile([128, 6, KC_B], bf16, tag="e")
                    gsb = g_t.rearrange("p (b s) -> p s b", s=6)
                    m_b = m.unsqueeze(1).broadcast_to([128, 6, KC_B])
                    nc.vector.tensor_tensor(e_t, gsb, m_b, op=Alu.is_ge)

                    # cols storage order: [col2, col1, col3, col0]
                    # col0=e3+e4+e5  col1=e1+e2+e5  col2=e0+e2+e4  col3=e0+e1+e3
                    ev = e_t  # [128, 6, 256]
                    s2 = msk.tile([128, 2, KC_B], bf16, tag="s2")
                    # s2 = [e0+e1, e4+e5]  (cheap on DVE; Pool's 0.42 eff loses)
                    nc.vector.tensor_add(s2, ev[:, 0::4, :], ev[:, 1::4, :])
                    t2 = msk.tile([128, 2, KC_B], bf16, tag="t2")
                    # t2 = [e0+e2, e1+e2]
                    eng.tensor_add(t2, ev[:, 0:2, :],
                                   ev[:, 2:3, :].broadcast_to([128, 2, KC_B]))
                    cols = msk.tile([128, 4, KC_B], bf16, tag="cols")
                    # [col2, col1] = t2 + [e4, e5]
                    eng.tensor_add(cols[:, 0:2, :], t2, ev[:, 4:6, :])
                    # [col3, col0] = s2 + e3
                    eng.tensor_add(cols[:, 2:4, :], s2,
                                   ev[:, 3:4, :].broadcast_to([128, 2, KC_B]))

                    w_t = w_tiles[ot]
                    # wm[o, s, kb] = w[o, s, kb] * col_s[o, kb]
                    # w packed with s-plane order (2, 1, 3, 0) to match cols
                    wm = w_pool.tile([128, 1024], bf16, tag="wm")
                    eng.tensor_mul(
                        wm, w_t, cols.rearrange("p s b -> p (s b)")
                    )

                    # transpose 8 subtiles [o, f] -> [f, o]; psum tile j holds
                    # f-range [j*128, (j+1)*128) (same map as host K_IDX)
                    ps = ps_xpose.tile([128, 8, 128], bf16, tag="psx")
                    wmv = wm.rearrange("p (j b) -> p j b", j=8)
                    for j in range(8):
                        nc.tensor.transpose(ps[:, j, :], wmv[:, j, :], ident)
                    nc.scalar.copy(
                        wmt[kc][:, :, ot * 128:(ot + 1) * 128], ps
                    )

                # xt prefetch at chunk boundaries
                for s_, h_ in XT_PLAN[kc]:
                    issue_xt(s_, h_)

            # ---------------- phase 2: GEMM ------------------------------
            for st in range(N_STRIP):
                issue_xt(st, 0); issue_xt(st, 1)
                accs = [ps_gemm.tile([128, O], f32, tag="acc", name=f"acc{st}_{i}")
                        for i in range(2)]
                early = st < N_EARLY and _os.environ.get("KV2_OSL", "1") == "1"
                act_out = (st < N_EARLY) or _os.environ.get("KV2_OUT") == "act"
                for h in range(2):
                    if act_out:
                        # bias as the zeroing first matmul; chunk-0 o-sliced
                        # so columns start as each ot's wmT copy lands
                        nc.tensor.matmul(accs[h], ones_t, bias_bf,
                                         start=True, stop=False,
                                         skip_group_check=True)
                    for kc in range(N_KC):
                        for j in range(8):
                            tl = (kc % 2) * 8 + j
                            lhs = xt_tiles[(st, kc // 2)][:, tl,
                                                          h * 128:(h + 1) * 128]
                            if early and kc == 0:
                                for ot in range(N_OT):
                                    osl = slice(ot * 128, (ot + 1) * 128)
                                    nc.tensor.matmul(
                                        accs[h][:, osl], lhs,
                                        wmt[kc][:, j, osl],
                                        start=False, stop=False,
                                        skip_group_check=True,
                                    )
                            else:
                                nc.tensor.matmul(
                                    accs[h], lhs, wmt[kc][:, j, :],
                                    start=(not act_out and kc == 0 and j == 0),
                                    stop=(kc == N_KC - 1 and j == 7),
                                    skip_group_check=act_out,
                                )
                for h in range(2):
                    o_t = out_pool.tile([128, O], bf16, tag="o",
                                        name=f"o_{st}_{h}")
                    if act_out:
                        nc.scalar.copy(o_t, accs[h])
                    else:
                        nc.vector.tensor_add(o_t, accs[h], bias_s)
                    nc.sync.dma_start(
                        out=out_d[st * TS + h * 128:st * TS + (h + 1) * 128, :],
                        in_=o_t,
                    )

    nc.compile()
    return nc


def _get_program(mode, const_c=None):
    key = mode
    if key not in _prog_cache:
        _prog_cache[key] = _build_program(mode)
    return _prog_cache[key]


# s-plane order for cols/w packing: cols tile holds [col2, col1, col3, col0]
PLANE_ORDER = (2, 1, 3, 0)


def _k_index():
    """K_IDX[tile, p] = source k for GEMM tile `tile`=kc*8+j, partition p.
    XBAR transpose of wm [128 o, 1024 f] lands f at out (p, j) = (f//8, f%8);
    f = i*256 + kb_local with cols-plane i -> k-offset PLANE_ORDER[i]."""
    tl = np.arange(32)[:, None]          # kc*8 + j
    p = np.arange(128)[None, :]
    kc = tl // 8
    f = (tl % 8) * 128 + p
    i = f // KC_B
    kb_local = f % KC_B
    s = np.asarray(PLANE_ORDER)[i]
    return 4 * (kc * KC_B + kb_local) + s


def _pack_host(x, weight, bias, gumbel_noise):
    import ml_dtypes
    bf16 = ml_dtypes.bfloat16

    x2 = np.asarray(x, np.float32).reshape(T, K).astype(bf16)
    kidx = _k_index().reshape(-1)
    # xt_packed[strip, p, tile, t] = x[strip*TS + t, K_IDX[tile, p]]
    xg = x2[:, kidx].reshape(N_STRIP, TS, 32, 128)
    xt_packed = np.ascontiguousarray(xg.transpose(0, 3, 2, 1)).reshape(
        N_STRIP * 128, 32 * TS
    )

    w = np.asarray(weight, np.float32).astype(bf16)
    b = np.asarray(bias, np.float32).reshape(1, O_FULL)
    g = np.asarray(gumbel_noise, np.float32).reshape(O_FULL, GUM_COLS)
    return xt_packed, w, b, g


def _pack_w_core(w_core):
    # w_packed[(kc, ot), po, s(plane order), kb]
    w6 = w_core.reshape(N_OT, 128, N_KC, KC_B, 4)  # [ot, po, kc, kb, s]
    w6 = w6[..., PLANE_ORDER]
    return np.ascontiguousarray(w6.transpose(2, 0, 1, 4, 3)).reshape(
        16 * 128, 1024
    )


def kernel(x, weight, bias, choice_weights, gumbel_noise):
    from concourse.bass_utils import run_bass_kernel_spmd

    cw = np.asarray(choice_weights, np.float32)
    c0 = float(cw.flat[0])
    is_const = bool((cw == c0).all())
    mode = "const" if is_const else "full"
    nc = _get_program(mode)

    xt_packed, w, b, g = _pack_host(x, weight, bias, gumbel_noise)

    in_maps = []
    for c in range(N_CORES):
        rows = slice(c * O, (c + 1) * O)
        m = {
            "xt": xt_packed,
            "w": _pack_w_core(w[rows]),
            "b": np.ascontiguousarray(b[:, rows]),
            "g": np.ascontiguousarray(g[rows]),
        }
        if mode == "full":
            m["cw"] = np.ascontiguousarray(cw.reshape(O_FULL, GUM_COLS)[rows])
        in_maps.append(m)

    res = run_bass_kernel_spmd(nc, in_maps, list(range(N_CORES)))
    parts = [np.asarray(res.results[c]["out"]).astype(np.float32)
             for c in range(N_CORES)]
    out = np.concatenate(parts, axis=1)  # [T, O_FULL]
    return out.reshape(2, 2048, O_FULL)


# revision 7
# speedup vs baseline: 1.0020x; 1.0020x over previous
"""Trainium2 Bass kernel for Gumbel 2:4-masked Linear (tensor-parallel over out_features).

Forward value (matches reference): mask = PATTERNS[argmax(cw + g, axis=-1)],
out = x @ (W * mask).T + b.  With constant choice_weights the argmax is
shift-invariant, so the mask is PATTERNS[argmax(g)].

v2 design (cost-model driven, walrus-ISA-legal):
  - bf16 GEMM (x, W, masked W): same PE rate as f32r in the cost model but
    half the DMA traffic and 2x DVE rate on mask math. Gumbel stays f32
    (bf16 would flip ~1e-3 of the argmaxes -> ~3e-2 error).
  - Mask build per (kc, ot) tile [128 o, 256 blocks]: one max-reduce and one
    batched is_ge (all 6 planes, broadcast max) on DVE (Pool's GPSIMD has no
    max/compare opcodes); 4 batched bf16 adds form the mask columns and one
    flat bf16 mul masks W -- those run on Pool for POOL_ITERS to offload DVE.
  - PE transposes masked-W subtiles via identity matmuls into PSUM; Act
    (otherwise idle) copies PSUM -> resident wmT tiles.
  - GEMM: 16 token strips x 2 PSUM chains of 32 accumulating matmuls over
    resident wmT; bias added during the DVE PSUM->SBUF copy (broadcast add);
    DMA out bf16, host upcasts to f32.
  - Host pre-packs xT/W (plane-split 2:4 layout, bf16) so every DMA moves
    >=2KB contiguous runs (full 360 GB/s in the model); per-strip xT
    descriptors are 8-16 KB.
  - xt strip loads interleaved into phase 1 at chunk boundaries so the GEMM
    starts as soon as wmt[0] lands; 7 PSUM acc slots + 1 transpose slot.
"""

import numpy as np

N_CORES = 8
T = 4096            # tokens
K = 4096            # in_features
O_FULL = 4096
O = O_FULL // N_CORES        # 512 out rows per core
B = K // 4                   # 1024 blocks per out row
GUM_COLS = B * 6             # 6144
N_KC = 4                     # k chunks
KC_B = B // N_KC             # 256 blocks per chunk
N_OT = O // 128              # 4 o-tiles
N_STRIP = 16                 # token strips
TS = T // N_STRIP            # 256 tokens per strip

import os as _os

# phase-1 iters (kc*2+kt, 0..7) whose adds / mul run on Pool (rest DVE)
POOL_ADDS = frozenset(
    int(x) for x in _os.environ.get("KV3_PA", "0,2,4,6").split(",") if x != "")
POOL_MULS = frozenset(
    int(x) for x in _os.environ.get("KV3_PM", "0,1,2,3,4,5,6,7").split(",")
    if x != "")
# strips using bias-matmul zeroing + o-sliced chunk-0 + Act-copy out
N_EARLY = int(_os.environ.get("KV2_EARLY", "0"))
# xt (strip, half) prefetches at each chunk boundary
_XT_PLANS = {
    "A": [[(0, 0), (1, 0)], [(2, 0), (0, 1)], [(1, 1), (3, 0), (2, 1)],
          [(3, 1)]],
    "B": [[(0, 0)], [(1, 0)], [(0, 1), (2, 0)], [(1, 1), (3, 0)]],
    "C": [[(0, 0)], [(1, 0), (2, 0)], [(0, 1), (1, 1)], [(2, 1), (3, 0)]],
    "D": [[(0, 0)], [(1, 0)], [(2, 0)], [(0, 1), (3, 0)]],
    "E": [[(0, 0)], [(1, 0), (2, 0)], [(3, 0), (0, 1), (4, 0)],
          [(1, 1), (5, 0)]],
    "F": [[(0, 0)], [(1, 0), (2, 0)], [(3, 0), (4, 0), (0, 1)],
          [(5, 0), (1, 1), (6, 0), (2, 1)]],
    "G": [[(0, 0), (1, 0)], [(2, 0), (3, 0)], [(4, 0), (0, 1), (5, 0)],
          [(1, 1), (6, 0), (2, 1)]],
}
XT_PLAN = _XT_PLANS[_os.environ.get("KV2_XT", "C")]

_prog_cache = {}


def _build_program(mode):
    """mode: 'const' (choice folded away) or 'full' (adds choice tensor)."""
    import concourse.bacc as bacc
    import concourse.bass as bass
    import concourse.mybir as mybir
    import concourse.tile as tile
    from concourse.masks import make_identity

    f32 = mybir.dt.float32
    bf16 = mybir.dt.bfloat16
    Alu = mybir.AluOpType

    nc = bacc.Bacc(trn_type="TRN2")
    # All mask math runs in TRANSPOSED (kb-partition) layout: each wmT GEMM
    # tile j = kbt*4 + i has partition p = kb_local and a fixed k-offset
    # s = PLANE_ORDER[i], so the masked transposed weight is built by pure
    # elementwise ops -- no PE transposes, no PSUM staging, no Act copies.
    # xt packed: [strip, p, (kc, kbt, i), t] -> [16*128, 32*256] bf16
    xt_d = nc.declare_dram_parameter("xt", [N_STRIP * 128, 32 * TS], bf16,
                                     isOutput=False)
    # wT packed: [(kc, kbt), p=kb_local, (i, o)] -> [8*128, 2048] bf16
    w_d = nc.declare_dram_parameter("w", [8 * 128, 2048], bf16, isOutput=False)
    b_d = nc.declare_dram_parameter("b", [1, O], f32, isOutput=False)
    # gumbel transposed: [kb, (o, s)] -> [1024, 512*6] f32
    g_d = nc.declare_dram_parameter("g", [B, O * 6], f32, isOutput=False)
    if mode == "full":
        cw_d = nc.declare_dram_parameter("cw", [B, O * 6], f32, isOutput=False)
    out_d = nc.declare_dram_parameter("out", [T, O], bf16, isOutput=True)

    with tile.TileContext(nc) as tc:
        with (
            tc.tile_pool(name="singles", bufs=1) as singles,
            tc.tile_pool(name="wmt", bufs=N_KC) as wmt_pool,
            tc.tile_pool(name="gum", bufs=3) as gum_pool,
            tc.tile_pool(name="wtile", bufs=4) as w_pool,
            tc.tile_pool(name="msk", bufs=2) as msk,
            tc.tile_pool(name="xth", bufs=8) as xt_pool,
            tc.tile_pool(name="outs", bufs=3) as out_pool,
            tc.tile_pool(name="ps_gemm", bufs=int(_os.environ.get("KV2_ACCB", "8")),
                         space="PSUM") as ps_gemm,
        ):
            bias_s = singles.tile([128, O], f32, name="bias_s")
            nc.gpsimd.dma_start(
                out=bias_s,
                in_=bass.AP(tensor=b_d, offset=0, ap=[[0, 128], [1, O]]),
            )
            bias_bf = singles.tile([1, O], bf16, name="bias_bf")
            nc.vector.tensor_copy(bias_bf, bias_s[0:1, :])
            ones_t = singles.tile([1, 128], bf16, name="ones")
            nc.vector.memset(ones_t, 1.0)

            # resident transposed masked weight, one tile per k chunk:
            # wmt[kc][kbp, j=(s,kbt), o]
            wmt = [
                wmt_pool.tile([128, 8, O], bf16, name=f"wmt{i}", tag=f"wmt{i}",
                              bufs=1)
                for i in range(N_KC)
            ]

            xt_tiles = {}  # (strip, half) -> tile

            def issue_xt(s, h):
                if (s, h) in xt_tiles:
                    return
                t_ = xt_pool.tile([128, 16, TS], bf16, tag="xth")
                nc.sync.dma_start(
                    out=t_,
                    in_=xt_d[s * 128:(s + 1) * 128,
                             h * 16 * TS:(h + 1) * 16 * TS],
                )
                xt_tiles[(s, h)] = t_

            # ---------------- phase 1: masks + masked W^T -----------------
            # 8 iterations (kc, kbt): [128 kb-part, 512 o] tiles.
            for kc in range(N_KC):
                g_tiles = [None] * 2
                w_tiles = [None] * 2

                def _load_g(kt, kc=kc, g_tiles=g_tiles):
                    rows = slice((kc * 2 + kt) * 128, (kc * 2 + kt + 1) * 128)
                    t_ = gum_pool.tile([128, O * 6], f32, tag="gum",
                                       name=f"g_{kc}_{kt}")
                    nc.sync.dma_start(out=t_, in_=g_d[rows, :])
                    g_tiles[kt] = t_

                def _load_w(kt, kc=kc, w_tiles=w_tiles):
                    rows = slice((kc * 2 + kt) * 128, (kc * 2 + kt + 1) * 128)
                    t_ = w_pool.tile([128, 2048], bf16, tag="w",
                                     name=f"w_{kc}_{kt}")
                    nc.sync.dma_start(out=t_, in_=w_d[rows, :])
                    w_tiles[kt] = t_

                if kc == 0:
                    _load_g(0); _load_w(0); _load_w(1); _load_g(1)
                else:
                    _load_g(0); _load_g(1); _load_w(0); _load_w(1)
                for kt in range(2):
                    it = kc * 2 + kt
                    # Pool is add/sub/mult-only on TRN2; reduce+compare are
                    # DVE-only. Pool takes adds/mul per POOL_ADDS/POOL_MULS.
                    eng_a = nc.gpsimd if it in POOL_ADDS else nc.vector
                    eng_m = nc.gpsimd if it in POOL_MULS else nc.vector
                    g_t = g_tiles[kt]
                    if mode == "full":
                        cw_t = gum_pool.tile([128, O * 6], f32, tag="cw",
                                             name=f"cw_{kc}_{kt}")
                        nc.sync.dma_start(
                            out=cw_t,
                            in_=cw_d[(kc * 2 + kt) * 128:
                                     (kc * 2 + kt + 1) * 128, :],
                        )
                        nc.vector.tensor_add(g_t, g_t, cw_t)
                    g3 = g_t.rearrange("p (o s) -> p o s", s=6)

                    m = msk.tile([128, O], f32, tag="m")
                    nc.vector.tensor_reduce(m, g3, axis=mybir.AxisListType.X,
                                            op=Alu.max)

                    # one-hot planes e[s][o] (multi-hot only on exact ties)
                    e_t = msk.tile([128, 6, O], bf16, tag="e")
                    gsb = g_t.rearrange("p (o s) -> p s o", s=6)
                    m_b = m.unsqueeze(1).broadcast_to([128, 6, O])
                    nc.vector.tensor_tensor(e_t, gsb, m_b, op=Alu.is_ge)

                    # cols storage order: [col2, col1, col3, col0]
                    # col0=e3+e4+e5  col1=e1+e2+e5  col2=e0+e2+e4  col3=e0+e1+e3
                    ev = e_t  # [128, 6, 512]
                    t2 = msk.tile([128, 2, O], bf16, tag="t2")
                    # t2 = [e0+e2, e1+e2]
                    eng_a.tensor_add(t2, ev[:, 0:2, :],
                                     ev[:, 2:3, :].broadcast_to([128, 2, O]))
                    cols = msk.tile([128, 4, O], bf16, tag="cols")
                    # [col2, col1] = t2 + [e4, e5]
                    eng_a.tensor_add(cols[:, 0:2, :], t2, ev[:, 4:6, :])
                    s2 = msk.tile([128, 2, O], bf16, tag="s2")
                    # s2 = [e0+e1, e4+e5]  (cheap on DVE; Pool's 0.42 eff loses)
                    nc.vector.tensor_add(s2, ev[:, 0::4, :], ev[:, 1::4, :])
                    # [col3, col0] = s2 + e3
                    eng_a.tensor_add(cols[:, 2:4, :], s2,
                                     ev[:, 3:4, :].broadcast_to([128, 2, O]))

                    # wmT tiles j = kt*4 + i, directly in SBUF:
                    # wmt[kc][p, j, o] = wT[p, i, o] * col_i[p, o]
                    eng_m.tensor_mul(
                        wmt[kc][:, kt * 4:(kt + 1) * 4, :].rearrange(
                            "p a b -> p (a b)"),
                        w_tiles[kt],
                        cols.rearrange("p s b -> p (s b)"),
                    )

                # xt prefetch at chunk boundaries
                for s_, h_ in XT_PLAN[kc]:
                    issue_xt(s_, h_)

            # ---------------- phase 2: GEMM ------------------------------
            for st in range(N_STRIP):
                issue_xt(st, 0); issue_xt(st, 1)
                accs = [ps_gemm.tile([128, O], f32, tag="acc", name=f"acc{st}_{i}")
                        for i in range(2)]
                early = st < N_EARLY and _os.environ.get("KV2_OSL", "1") == "1"
                act_out = (st < N_EARLY) or _os.environ.get("KV2_OUT") == "act"
                for h in range(2):
                    if act_out:
                        # bias as the zeroing first matmul; chunk-0 o-sliced
                        # so columns start as each ot's wmT copy lands
                        nc.tensor.matmul(accs[h], ones_t, bias_bf,
                                         start=True, stop=False,
                                         skip_group_check=True)
                    for kc in range(N_KC):
                        for j in range(8):
                            tl = (kc % 2) * 8 + j
                            lhs = xt_tiles[(st, kc // 2)][:, tl,
                                                          h * 128:(h + 1) * 128]
                            if early and kc == 0:
                                for ot in range(N_OT):
                                    osl = slice(ot * 128, (ot + 1) * 128)
                                    nc.tensor.matmul(
                                        accs[h][:, osl], lhs,
                                        wmt[kc][:, j, osl],
                                        start=False, stop=False,
                                        skip_group_check=True,
                                    )
                            else:
                                nc.tensor.matmul(
                                    accs[h], lhs, wmt[kc][:, j, :],
                                    start=(not act_out and kc == 0 and j == 0),
                                    stop=(kc == N_KC - 1 and j == 7),
                                    skip_group_check=act_out,
                                )
                for h in range(2):
                    o_t = out_pool.tile([128, O], bf16, tag="o",
                                        name=f"o_{st}_{h}")
                    if act_out:
                        nc.scalar.copy(o_t, accs[h])
                    else:
                        nc.vector.tensor_add(o_t, accs[h], bias_s)
                    nc.sync.dma_start(
                        out=out_d[st * TS + h * 128:st * TS + (h + 1) * 128, :],
                        in_=o_t,
                    )

    nc.compile()
    return nc


def _get_program(mode, const_c=None):
    key = mode
    if key not in _prog_cache:
        _prog_cache[key] = _build_program(mode)
    return _prog_cache[key]


# s-plane order for cols/w packing: cols tile holds [col2, col1, col3, col0]
PLANE_ORDER = (2, 1, 3, 0)


def _k_index():
    """K_IDX[tile, p] = source k for GEMM tile `tile`=kc*8+j, partition p.
    Tile j = kbt*4 + i: partition p = kb_local (block within the kbt-half of
    chunk kc), fixed k-offset s = PLANE_ORDER[i] per tile."""
    tl = np.arange(32)[:, None]          # kc*8 + j
    p = np.arange(128)[None, :]
    kc = tl // 8
    j = tl % 8
    kbt = j // 4
    i = j % 4
    s = np.asarray(PLANE_ORDER)[i]
    return 4 * (kc * KC_B + kbt * 128 + p) + s


def _pack_host(x, weight, bias, gumbel_noise):
    import ml_dtypes
    bf16 = ml_dtypes.bfloat16

    x2 = np.asarray(x, np.float32).reshape(T, K).astype(bf16)
    kidx = _k_index().reshape(-1)
    # xt_packed[strip, p, tile, t] = x[strip*TS + t, K_IDX[tile, p]]
    xg = x2[:, kidx].reshape(N_STRIP, TS, 32, 128)
    xt_packed = np.ascontiguousarray(xg.transpose(0, 3, 2, 1)).reshape(
        N_STRIP * 128, 32 * TS
    )

    w = np.asarray(weight, np.float32).astype(bf16)
    b = np.asarray(bias, np.float32).reshape(1, O_FULL)
    g = np.asarray(gumbel_noise, np.float32).reshape(O_FULL, GUM_COLS)
    return xt_packed, w, b, g


def _pack_w_core(w_core):
    """wT_packed[(kc,kbt)*128 + p, i*512 + o] = w_core[o, K_IDX[kc*8+kbt*4+i, p]]"""
    wT = np.ascontiguousarray(w_core.T)              # [K, O] bf16
    kidx = _k_index().reshape(N_KC, 2, 4, 128)       # [kc, kbt, i, p]
    rows = wT[kidx]                                  # [kc, kbt, i, p, O]
    return np.ascontiguousarray(
        rows.transpose(0, 1, 3, 2, 4)).reshape(8 * 128, 4 * O)


def _pack_g_core(g_core):
    """gT_packed[kb, o*6 + s] from g_core[o, kb*6 + s]."""
    g3 = g_core.reshape(O, B, 6)
    return np.ascontiguousarray(g3.transpose(1, 0, 2)).reshape(B, O * 6)


def kernel(x, weight, bias, choice_weights, gumbel_noise):
    from concourse.bass_utils import run_bass_kernel_spmd

    cw = np.asarray(choice_weights, np.float32)
    c0 = float(cw.flat[0])
    is_const = bool((cw == c0).all())
    mode = "const" if is_const else "full"
    nc = _get_program(mode)

    xt_packed, w, b, g = _pack_host(x, weight, bias, gumbel_noise)

    in_maps = []
    for c in range(N_CORES):
        rows = slice(c * O, (c + 1) * O)
        m = {
            "xt": xt_packed,
            "w": _pack_w_core(w[rows]),
            "b": np.ascontiguousarray(b[:, rows]),
            "g": _pack_g_core(g[rows]),
        }
        if mode == "full":
            m["cw"] = _pack_g_core(cw.reshape(O_FULL, GUM_COLS)[rows])
        in_maps.append(m)

    res = run_bass_kernel_spmd(nc, in_maps, list(range(N_CORES)))
    parts = [np.asarray(res.results[c]["out"]).astype(np.float32)
             for c in range(N_CORES)]
    out = np.concatenate(parts, axis=1)  # [T, O_FULL]
    return out.reshape(2, 2048, O_FULL)


# revision 9
# speedup vs baseline: 1.0045x; 1.0026x over previous
"""Trainium2 Bass kernel for Gumbel 2:4-masked Linear (tensor-parallel over out_features).

Forward value (matches reference): mask = PATTERNS[argmax(cw + g, axis=-1)],
out = x @ (W * mask).T + b.  With constant choice_weights the argmax is
shift-invariant, so the mask is PATTERNS[argmax(g)].

v2 design (cost-model driven, walrus-ISA-legal):
  - bf16 GEMM (x, W, masked W): same PE rate as f32r in the cost model but
    half the DMA traffic and 2x DVE rate on mask math. Gumbel stays f32
    (bf16 would flip ~1e-3 of the argmaxes -> ~3e-2 error).
  - Mask build per (kc, ot) tile [128 o, 256 blocks]: one max-reduce and one
    batched is_ge (all 6 planes, broadcast max) on DVE (Pool's GPSIMD has no
    max/compare opcodes); 4 batched bf16 adds form the mask columns and one
    flat bf16 mul masks W -- those run on Pool for POOL_ITERS to offload DVE.
  - v3: all mask math runs in TRANSPOSED (kb-partition) layout -- each wmT
    GEMM tile has partition = block index with a fixed k-offset, so the
    host-transposed gumbel/W let one flat mul write wmT directly to SBUF:
    no PE transposes, no PSUM staging, no Act copies, 8 PSUM GEMM slots.
  - GEMM: 16 token strips x 2 PSUM chains of 32 accumulating matmuls over
    resident wmT; bias added during the DVE PSUM->SBUF copy (broadcast add);
    DMA out bf16, host upcasts to f32.
  - Host pre-packs xT/W (plane-split 2:4 layout, bf16) so every DMA moves
    >=2KB contiguous runs (full 360 GB/s in the model); per-strip xT
    descriptors are 8-16 KB.
  - xt strip loads interleaved into phase 1 at chunk boundaries so the GEMM
    starts as soon as wmt[0] lands; 7 PSUM acc slots + 1 transpose slot.
"""

import numpy as np

N_CORES = 8
T = 4096            # tokens
K = 4096            # in_features
O_FULL = 4096
O = O_FULL // N_CORES        # 512 out rows per core
B = K // 4                   # 1024 blocks per out row
GUM_COLS = B * 6             # 6144
N_KC = 4                     # k chunks
KC_B = B // N_KC             # 256 blocks per chunk
N_OT = O // 128              # 4 o-tiles
N_STRIP = 16                 # token strips
TS = T // N_STRIP            # 256 tokens per strip

import os as _os

# phase-1 iters (kc*2+kt, 0..7) whose adds / mul run on Pool (rest DVE)
POOL_ADDS = frozenset(
    int(x) for x in _os.environ.get("KV3_PA", "0,2,4,6").split(",") if x != "")
POOL_MULS = frozenset(
    int(x) for x in _os.environ.get("KV3_PM", "0,1,2,3,4,5,6,7").split(",")
    if x != "")
# strips using bias-matmul zeroing + o-sliced chunk-0 + Act-copy out
N_EARLY = int(_os.environ.get("KV2_EARLY", "0"))
# xt (strip, half) prefetches at each chunk boundary
_XT_PLANS = {
    "A": [[(0, 0), (1, 0)], [(2, 0), (0, 1)], [(1, 1), (3, 0), (2, 1)],
          [(3, 1)]],
    "B": [[(0, 0)], [(1, 0)], [(0, 1), (2, 0)], [(1, 1), (3, 0)]],
    "C": [[(0, 0)], [(1, 0), (2, 0)], [(0, 1), (1, 1)], [(2, 1), (3, 0)]],
    "D": [[(0, 0)], [(1, 0)], [(2, 0)], [(0, 1), (3, 0)]],
    "E": [[(0, 0)], [(1, 0), (2, 0)], [(3, 0), (0, 1), (4, 0)],
          [(1, 1), (5, 0)]],
    "F": [[(0, 0)], [(1, 0), (2, 0)], [(3, 0), (4, 0), (0, 1)],
          [(5, 0), (1, 1), (6, 0), (2, 1)]],
    "G": [[(0, 0), (1, 0)], [(2, 0), (3, 0)], [(4, 0), (0, 1), (5, 0)],
          [(1, 1), (6, 0), (2, 1)]],
}
XT_PLAN = _XT_PLANS[_os.environ.get("KV2_XT", "C")]

_prog_cache = {}


def _build_program(mode):
    """mode: 'const' (choice folded away) or 'full' (adds choice tensor)."""
    import concourse.bacc as bacc
    import concourse.bass as bass
    import concourse.mybir as mybir
    import concourse.tile as tile
    from concourse.masks import make_identity

    f32 = mybir.dt.float32
    bf16 = mybir.dt.bfloat16
    Alu = mybir.AluOpType

    nc = bacc.Bacc(trn_type="TRN2")
    # All mask math runs in TRANSPOSED (kb-partition) layout: each wmT GEMM
    # tile j = kbt*4 + i has partition p = kb_local and a fixed k-offset
    # s = PLANE_ORDER[i], so the masked transposed weight is built by pure
    # elementwise ops -- no PE transposes, no PSUM staging, no Act copies.
    # xt packed: [strip, p, (kc, kbt, i), t] -> [16*128, 32*256] bf16
    xt_d = nc.declare_dram_parameter("xt", [N_STRIP * 128, 32 * TS], bf16,
                                     isOutput=False)
    # wT packed: [(kc, kbt), p=kb_local, (i, o)] -> [8*128, 2048] bf16
    w_d = nc.declare_dram_parameter("w", [8 * 128, 2048], bf16, isOutput=False)
    b_d = nc.declare_dram_parameter("b", [1, O], f32, isOutput=False)
    # gumbel transposed: [kb, (o, s)] -> [1024, 512*6] f32
    g_d = nc.declare_dram_parameter("g", [B, O * 6], f32, isOutput=False)
    if mode == "full":
        cw_d = nc.declare_dram_parameter("cw", [B, O * 6], f32, isOutput=False)
    out_d = nc.declare_dram_parameter("out", [T, O], bf16, isOutput=True)

    with tile.TileContext(nc) as tc:
        with (
            tc.tile_pool(name="singles", bufs=1) as singles,
            tc.tile_pool(name="wmt", bufs=N_KC) as wmt_pool,
            tc.tile_pool(name="gum", bufs=3) as gum_pool,
            tc.tile_pool(name="wtile", bufs=4) as w_pool,
            tc.tile_pool(name="msk", bufs=2) as msk,
            tc.tile_pool(name="xth", bufs=8) as xt_pool,
            tc.tile_pool(name="outs", bufs=3) as out_pool,
            tc.tile_pool(name="ps_gemm", bufs=int(_os.environ.get("KV2_ACCB", "7")),
                         space="PSUM") as ps_gemm,
            tc.tile_pool(name="ps_warm", bufs=1, space="PSUM") as ps_warm,
        ):
            bias_s = singles.tile([128, O], f32, name="bias_s")
            nc.gpsimd.dma_start(
                out=bias_s,
                in_=bass.AP(tensor=b_d, offset=0, ap=[[0, 128], [1, O]]),
            )
            bias_bf = singles.tile([1, O], bf16, name="bias_bf")
            nc.vector.tensor_copy(bias_bf, bias_s[0:1, :])
            ones_t = singles.tile([1, 128], bf16, name="ones")
            nc.vector.memset(ones_t, 1.0)

            # resident transposed masked weight, one tile per k chunk:
            # wmt[kc][kbp, j=(s,kbt), o]
            wmt = [
                wmt_pool.tile([128, 8, O], bf16, name=f"wmt{i}", tag=f"wmt{i}",
                              bufs=1)
                for i in range(N_KC)
            ]

            xt_tiles = {}  # (strip, half) -> tile

            def issue_xt(s, h):
                if (s, h) in xt_tiles:
                    return
                t_ = xt_pool.tile([128, 16, TS], bf16, tag="xth")
                nc.sync.dma_start(
                    out=t_,
                    in_=xt_d[s * 128:(s + 1) * 128,
                             h * 16 * TS:(h + 1) * 16 * TS],
                )
                xt_tiles[(s, h)] = t_

            # ---------------- phase 1: masks + masked W^T -----------------
            # 8 iterations (kc, kbt): [128 kb-part, 512 o] tiles.
            for kc in range(N_KC):
                g_tiles = [None] * 2
                w_tiles = [None] * 2

                def _load_g(kt, kc=kc, g_tiles=g_tiles):
                    rows = slice((kc * 2 + kt) * 128, (kc * 2 + kt + 1) * 128)
                    t_ = gum_pool.tile([128, O * 6], f32, tag="gum",
                                       name=f"g_{kc}_{kt}")
                    nc.sync.dma_start(out=t_, in_=g_d[rows, :])
                    g_tiles[kt] = t_

                def _load_w(kt, kc=kc, w_tiles=w_tiles):
                    rows = slice((kc * 2 + kt) * 128, (kc * 2 + kt + 1) * 128)
                    t_ = w_pool.tile([128, 2048], bf16, tag="w",
                                     name=f"w_{kc}_{kt}")
                    nc.sync.dma_start(out=t_, in_=w_d[rows, :])
                    w_tiles[kt] = t_

                if kc == 0:
                    _load_g(0); _load_w(0); _load_w(1); _load_g(1)
                else:
                    _load_g(0); _load_g(1); _load_w(0); _load_w(1)
                for kt in range(2):
                    it = kc * 2 + kt
                    # Pool is add/sub/mult-only on TRN2; reduce+compare are
                    # DVE-only. Pool takes adds/mul per POOL_ADDS/POOL_MULS.
                    eng_a = nc.gpsimd if it in POOL_ADDS else nc.vector
                    eng_m = nc.gpsimd if it in POOL_MULS else nc.vector
                    g_t = g_tiles[kt]
                    if mode == "full":
                        cw_t = gum_pool.tile([128, O * 6], f32, tag="cw",
                                             name=f"cw_{kc}_{kt}")
                        nc.sync.dma_start(
                            out=cw_t,
                            in_=cw_d[(kc * 2 + kt) * 128:
                                     (kc * 2 + kt + 1) * 128, :],
                        )
                        nc.vector.tensor_add(g_t, g_t, cw_t)
                    g3 = g_t.rearrange("p (o s) -> p o s", s=6)

                    m = msk.tile([128, O], f32, tag="m")
                    nc.vector.tensor_reduce(m, g3, axis=mybir.AxisListType.X,
                                            op=Alu.max)

                    # one-hot planes e[s][o] (multi-hot only on exact ties)
                    e_t = msk.tile([128, 6, O], bf16, tag="e")
                    gsb = g_t.rearrange("p (o s) -> p s o", s=6)
                    m_b = m.unsqueeze(1).broadcast_to([128, 6, O])
                    nc.vector.tensor_tensor(e_t, gsb, m_b, op=Alu.is_ge)

                    # cols storage order: [col2, col1, col3, col0]
                    # col0=e3+e4+e5  col1=e1+e2+e5  col2=e0+e2+e4  col3=e0+e1+e3
                    ev = e_t  # [128, 6, 512]
                    t2 = msk.tile([128, 2, O], bf16, tag="t2")
                    # t2 = [e0+e2, e1+e2]
                    eng_a.tensor_add(t2, ev[:, 0:2, :],
                                     ev[:, 2:3, :].broadcast_to([128, 2, O]))
                    cols = msk.tile([128, 4, O], bf16, tag="cols")
                    # [col2, col1] = t2 + [e4, e5]
                    eng_a.tensor_add(cols[:, 0:2, :], t2, ev[:, 4:6, :])
                    s2 = msk.tile([128, 2, O], bf16, tag="s2")
                    # s2 = [e0+e1, e4+e5]  (cheap on DVE; Pool's 0.42 eff loses)
                    nc.vector.tensor_add(s2, ev[:, 0::4, :], ev[:, 1::4, :])
                    # [col3, col0] = s2 + e3
                    eng_a.tensor_add(cols[:, 2:4, :], s2,
                                     ev[:, 3:4, :].broadcast_to([128, 2, O]))

                    # wmT tiles j = kt*4 + i, directly in SBUF:
                    # wmt[kc][p, j, o] = wT[p, i, o] * col_i[p, o]
                    eng_m.tensor_mul(
                        wmt[kc][:, kt * 4:(kt + 1) * 4, :].rearrange(
                            "p a b -> p (a b)"),
                        w_tiles[kt],
                        cols.rearrange("p s b -> p (s b)"),
                    )

                # xt prefetch at chunk boundaries
                for s_, h_ in XT_PLAN[kc]:
                    issue_xt(s_, h_)

            # ---------------- phase 2: GEMM ------------------------------
            for st in range(N_STRIP):
                issue_xt(st, 0); issue_xt(st, 1)
                accs = [ps_gemm.tile([128, O], f32, tag="acc", name=f"acc{st}_{i}")
                        for i in range(2)]
                early = st < N_EARLY and _os.environ.get("KV2_OSL", "1") == "1"
                act_out = (st < N_EARLY) or _os.environ.get("KV2_OUT") == "act"
                for h in range(2):
                    if act_out:
                        # bias as the zeroing first matmul; chunk-0 o-sliced
                        # so columns start as each ot's wmT copy lands
                        nc.tensor.matmul(accs[h], ones_t, bias_bf,
                                         start=True, stop=False,
                                         skip_group_check=True)
                    for kc in range(N_KC):
                        for j in range(8):
                            tl = (kc % 2) * 8 + j
                            lhs = xt_tiles[(st, kc // 2)][:, tl,
                                                          h * 128:(h + 1) * 128]
                            if early and kc == 0:
                                for ot in range(N_OT):
                                    osl = slice(ot * 128, (ot + 1) * 128)
                                    nc.tensor.matmul(
                                        accs[h][:, osl], lhs,
                                        wmt[kc][:, j, osl],
                                        start=False, stop=False,
                                        skip_group_check=True,
                                    )
                            else:
                                nc.tensor.matmul(
                                    accs[h], lhs, wmt[kc][:, j, :],
                                    start=(not act_out and kc == 0 and j == 0),
                                    stop=(kc == N_KC - 1 and j == 7),
                                    skip_group_check=act_out,
                                )
                for h in range(2):
                    o_t = out_pool.tile([128, O], bf16, tag="o",
                                        name=f"o_{st}_{h}")
                    if act_out:
                        nc.scalar.copy(o_t, accs[h])
                    else:
                        nc.vector.tensor_add(o_t, accs[h], bias_s)
                    nc.sync.dma_start(
                        out=out_d[st * TS + h * 128:st * TS + (h + 1) * 128, :],
                        in_=o_t,
                    )

            # PE p-state warmers: lowest-priority (program-end) rank-1
            # matmuls into a scratch bank. They run only when PE is idle
            # (the cold phase-1 window), keeping the ramp at full speed for
            # the real GEMM. Count must drain within the early idle.
            n_warm = int(_os.environ.get("KV3_WARM", "120"))
            if n_warm:
                scratch = ps_warm.tile([128, O], f32, name="scratch")
                for wi in range(n_warm):
                    nc.tensor.matmul(scratch, ones_t, bias_bf,
                                     start=True, stop=True,
                                     skip_group_check=True)

    nc.compile()
    return nc


def _get_program(mode, const_c=None):
    key = mode
    if key not in _prog_cache:
        _prog_cache[key] = _build_program(mode)
    return _prog_cache[key]


# s-plane order for cols/w packing: cols tile holds [col2, col1, col3, col0]
PLANE_ORDER = (2, 1, 3, 0)


def _k_index():
    """K_IDX[tile, p] = source k for GEMM tile `tile`=kc*8+j, partition p.
    Tile j = kbt*4 + i: partition p = kb_local (block within the kbt-half of
    chunk kc), fixed k-offset s = PLANE_ORDER[i] per tile."""
    tl = np.arange(32)[:, None]          # kc*8 + j
    p = np.arange(128)[None, :]
    kc = tl // 8
    j = tl % 8
    kbt = j // 4
    i = j % 4
    s = np.asarray(PLANE_ORDER)[i]
    return 4 * (kc * KC_B + kbt * 128 + p) + s


def _pack_host(x, weight, bias, gumbel_noise):
    import ml_dtypes
    bf16 = ml_dtypes.bfloat16

    x2 = np.asarray(x, np.float32).reshape(T, K).astype(bf16)
    kidx = _k_index().reshape(-1)
    # xt_packed[strip, p, tile, t] = x[strip*TS + t, K_IDX[tile, p]]
    xg = x2[:, kidx].reshape(N_STRIP, TS, 32, 128)
    xt_packed = np.ascontiguousarray(xg.transpose(0, 3, 2, 1)).reshape(
        N_STRIP * 128, 32 * TS
    )

    w = np.asarray(weight, np.float32).astype(bf16)
    b = np.asarray(bias, np.float32).reshape(1, O_FULL)
    g = np.asarray(gumbel_noise, np.float32).reshape(O_FULL, GUM_COLS)
    return xt_packed, w, b, g


def _pack_w_core(w_core):
    """wT_packed[(kc,kbt)*128 + p, i*512 + o] = w_core[o, K_IDX[kc*8+kbt*4+i, p]]"""
    wT = np.ascontiguousarray(w_core.T)              # [K, O] bf16
    kidx = _k_index().reshape(N_KC, 2, 4, 128)       # [kc, kbt, i, p]
    rows = wT[kidx]                                  # [kc, kbt, i, p, O]
    return np.ascontiguousarray(
        rows.transpose(0, 1, 3, 2, 4)).reshape(8 * 128, 4 * O)


def _pack_g_core(g_core):
    """gT_packed[kb, o*6 + s] from g_core[o, kb*6 + s]."""
    g3 = g_core.reshape(O, B, 6)
    return np.ascontiguousarray(g3.transpose(1, 0, 2)).reshape(B, O * 6)


def kernel(x, weight, bias, choice_weights, gumbel_noise):
    from concourse.bass_utils import run_bass_kernel_spmd

    cw = np.asarray(choice_weights, np.float32)
    c0 = float(cw.flat[0])
    is_const = bool((cw == c0).all())
    mode = "const" if is_const else "full"
    nc = _get_program(mode)

    xt_packed, w, b, g = _pack_host(x, weight, bias, gumbel_noise)

    in_maps = []
    for c in range(N_CORES):
        rows = slice(c * O, (c + 1) * O)
        m = {
            "xt": xt_packed,
            "w": _pack_w_core(w[rows]),
            "b": np.ascontiguousarray(b[:, rows]),
            "g": _pack_g_core(g[rows]),
        }
        if mode == "full":
            m["cw"] = _pack_g_core(cw.reshape(O_FULL, GUM_COLS)[rows])
        in_maps.append(m)

    res = run_bass_kernel_spmd(nc, in_maps, list(range(N_CORES)))
    parts = [np.asarray(res.results[c]["out"]).astype(np.float32)
             for c in range(N_CORES)]
    out = np.concatenate(parts, axis=1)  # [T, O_FULL]
    return out.reshape(2, 2048, O_FULL)


# revision 12
# speedup vs baseline: 1.0116x; 1.0070x over previous
"""Trainium2 Bass kernel for Gumbel 2:4-masked Linear (tensor-parallel over out_features).

Forward value (matches reference): mask = PATTERNS[argmax(cw + g, axis=-1)],
out = x @ (W * mask).T + b.  With constant choice_weights the argmax is
shift-invariant, so the mask is PATTERNS[argmax(g)].

v2 design (cost-model driven, walrus-ISA-legal):
  - bf16 GEMM (x, W, masked W): same PE rate as f32r in the cost model but
    half the DMA traffic and 2x DVE rate on mask math. Gumbel stays f32
    (bf16 would flip ~1e-3 of the argmaxes -> ~3e-2 error).
  - Mask build per (kc, ot) tile [128 o, 256 blocks]: one max-reduce and one
    batched is_ge (all 6 planes, broadcast max) on DVE (Pool's GPSIMD has no
    max/compare opcodes); 4 batched bf16 adds form the mask columns and one
    flat bf16 mul masks W -- those run on Pool for POOL_ITERS to offload DVE.
  - v3: all mask math runs in TRANSPOSED (kb-partition) layout -- each wmT
    GEMM tile has partition = block index with a fixed k-offset, so the
    host-transposed gumbel/W let one flat mul write wmT directly to SBUF:
    no PE transposes, no PSUM staging, no Act copies, 8 PSUM GEMM slots.
  - GEMM: 16 token strips x 2 PSUM chains of 32 accumulating matmuls over
    resident wmT; bias added during the DVE PSUM->SBUF copy (broadcast add);
    DMA out bf16, host upcasts to f32.
  - Host pre-packs xT/W (plane-split 2:4 layout, bf16) so every DMA moves
    >=2KB contiguous runs (full 360 GB/s in the model); per-strip xT
    descriptors are 8-16 KB.
  - xt strip loads interleaved into phase 1 at chunk boundaries so the GEMM
    starts as soon as wmt[0] lands; 7 PSUM acc slots + 1 transpose slot.
"""

import numpy as np

N_CORES = 8
T = 4096            # tokens
K = 4096            # in_features
O_FULL = 4096
O = O_FULL // N_CORES        # 512 out rows per core
B = K // 4                   # 1024 blocks per out row
GUM_COLS = B * 6             # 6144
N_KC = 4                     # k chunks
KC_B = B // N_KC             # 256 blocks per chunk
N_OT = O // 128              # 4 o-tiles
N_STRIP = 16                 # token strips
TS = T // N_STRIP            # 256 tokens per strip

import os as _os

# phase-1 iters (kc*2+kt, 0..7) whose adds / mul run on Pool (rest DVE)
POOL_ADDS = frozenset(
    int(x) for x in _os.environ.get("KV3_PA", "0,2,4,6").split(",") if x != "")
POOL_MULS = frozenset(
    int(x) for x in _os.environ.get("KV3_PM", "0,1,2,3,5,7").split(",")
    if x != "")
# strips using bias-matmul zeroing + o-sliced chunk-0 + Act-copy out
N_EARLY = int(_os.environ.get("KV2_EARLY", "0"))
# xt (strip, half) prefetches at each chunk boundary
_XT_PLANS = {
    "A": [[(0, 0), (1, 0)], [(2, 0), (0, 1)], [(1, 1), (3, 0), (2, 1)],
          [(3, 1)]],
    "B": [[(0, 0)], [(1, 0)], [(0, 1), (2, 0)], [(1, 1), (3, 0)]],
    "C": [[(0, 0)], [(1, 0), (2, 0)], [(0, 1), (1, 1)], [(2, 1), (3, 0)]],
    "D": [[(0, 0)], [(1, 0)], [(2, 0)], [(0, 1), (3, 0)]],
    "E": [[(0, 0)], [(1, 0), (2, 0)], [(3, 0), (0, 1), (4, 0)],
          [(1, 1), (5, 0)]],
    "F": [[(0, 0)], [(1, 0), (2, 0)], [(3, 0), (4, 0), (0, 1)],
          [(5, 0), (1, 1), (6, 0), (2, 1)]],
    "G": [[(0, 0), (1, 0)], [(2, 0), (3, 0)], [(4, 0), (0, 1), (5, 0)],
          [(1, 1), (6, 0), (2, 1)]],
    "H": [[(0, 0), (1, 0)], [(2, 0), (0, 1)], [(1, 1), (3, 0), (2, 1)],
          [(3, 1), (4, 0)]],
    "I": [[(0, 0), (1, 0)], [(2, 0), (3, 0)], [(0, 1), (1, 1), (2, 1)],
          [(3, 1)]],
}
XT_PLAN = _XT_PLANS[_os.environ.get("KV2_XT", "A")]

_prog_cache = {}


def _build_program(mode):
    """mode: 'const' (choice folded away) or 'full' (adds choice tensor)."""
    import concourse.bacc as bacc
    import concourse.bass as bass
    import concourse.mybir as mybir
    import concourse.tile as tile
    from concourse.masks import make_identity

    f32 = mybir.dt.float32
    bf16 = mybir.dt.bfloat16
    Alu = mybir.AluOpType

    nc = bacc.Bacc(trn_type="TRN2")
    # All mask math runs in TRANSPOSED (kb-partition) layout: each wmT GEMM
    # tile j = kbt*4 + i has partition p = kb_local and a fixed k-offset
    # s = PLANE_ORDER[i], so the masked transposed weight is built by pure
    # elementwise ops -- no PE transposes, no PSUM staging, no Act copies.
    # xt packed: [strip, p, (kc, kbt, i), t] -> [16*128, 32*256] bf16
    xt_d = nc.declare_dram_parameter("xt", [N_STRIP * 128, 32 * TS], bf16,
                                     isOutput=False)
    # wT packed: [(kc, kbt), p=kb_local, (i, o)] -> [8*128, 2048] bf16
    w_d = nc.declare_dram_parameter("w", [8 * 128, 2048], bf16, isOutput=False)
    b_d = nc.declare_dram_parameter("b", [1, O], f32, isOutput=False)
    # gumbel transposed: [kb, (o, s)] -> [1024, 512*6] f32
    g_d = nc.declare_dram_parameter("g", [B, O * 6], f32, isOutput=False)
    if mode == "full":
        cw_d = nc.declare_dram_parameter("cw", [B, O * 6], f32, isOutput=False)
    out_d = nc.declare_dram_parameter("out", [T, O], bf16, isOutput=True)

    with tile.TileContext(nc) as tc:
        with (
            tc.tile_pool(name="singles", bufs=1) as singles,
            tc.tile_pool(name="wmt", bufs=N_KC) as wmt_pool,
            tc.tile_pool(name="gum", bufs=3) as gum_pool,
            tc.tile_pool(name="wtile", bufs=4) as w_pool,
            tc.tile_pool(name="msk", bufs=2) as msk,
            tc.tile_pool(name="xth", bufs=8) as xt_pool,
            tc.tile_pool(name="outs", bufs=3) as out_pool,
            tc.tile_pool(name="ps_gemm", bufs=int(_os.environ.get("KV2_ACCB", "7")),
                         space="PSUM") as ps_gemm,
            tc.tile_pool(name="ps_warm", bufs=1, space="PSUM") as ps_warm,
        ):
            bias_s = singles.tile([128, O], f32, name="bias_s")
            nc.gpsimd.dma_start(
                out=bias_s,
                in_=bass.AP(tensor=b_d, offset=0, ap=[[0, 128], [1, O]]),
            )
            bias_bf = singles.tile([1, O], bf16, name="bias_bf")
            nc.vector.tensor_copy(bias_bf, bias_s[0:1, :])
            ones_t = singles.tile([1, 128], bf16, name="ones")
            nc.vector.memset(ones_t, 1.0)

            # resident transposed masked weight, one tile per k chunk:
            # wmt[kc][kbp, j=(s,kbt), o]
            wmt = [
                wmt_pool.tile([128, 8, O], bf16, name=f"wmt{i}", tag=f"wmt{i}",
                              bufs=1)
                for i in range(N_KC)
            ]

            xt_tiles = {}  # (strip, half) -> tile

            def issue_xt(s, h):
                if (s, h) in xt_tiles:
                    return
                t_ = xt_pool.tile([128, 16, TS], bf16, tag="xth")
                nc.sync.dma_start(
                    out=t_,
                    in_=xt_d[s * 128:(s + 1) * 128,
                             h * 16 * TS:(h + 1) * 16 * TS],
                )
                xt_tiles[(s, h)] = t_

            # ---------------- phase 1: masks + masked W^T -----------------
            # 8 iterations (kc, kbt): [128 kb-part, 512 o] tiles.
            for kc in range(N_KC):
                g_tiles = [None] * 2
                w_tiles = [None] * 2

                def _load_g(kt, kc=kc, g_tiles=g_tiles):
                    rows = slice((kc * 2 + kt) * 128, (kc * 2 + kt + 1) * 128)
                    t_ = gum_pool.tile([128, O * 6], f32, tag="gum",
                                       name=f"g_{kc}_{kt}")
                    nc.sync.dma_start(out=t_, in_=g_d[rows, :])
                    g_tiles[kt] = t_

                def _load_w(kt, kc=kc, w_tiles=w_tiles):
                    rows = slice((kc * 2 + kt) * 128, (kc * 2 + kt + 1) * 128)
                    t_ = w_pool.tile([128, 2048], bf16, tag="w",
                                     name=f"w_{kc}_{kt}")
                    nc.sync.dma_start(out=t_, in_=w_d[rows, :])
                    w_tiles[kt] = t_

                if kc == 0:
                    _load_g(0); _load_w(0); _load_w(1); _load_g(1)
                else:
                    _load_g(0); _load_g(1); _load_w(0); _load_w(1)
                for kt in range(2):
                    it = kc * 2 + kt
                    # Pool is add/sub/mult-only on TRN2; reduce+compare are
                    # DVE-only. Pool takes adds/mul per POOL_ADDS/POOL_MULS.
                    eng_a = nc.gpsimd if it in POOL_ADDS else nc.vector
                    eng_m = nc.gpsimd if it in POOL_MULS else nc.vector
                    g_t = g_tiles[kt]
                    if mode == "full":
                        cw_t = gum_pool.tile([128, O * 6], f32, tag="cw",
                                             name=f"cw_{kc}_{kt}")
                        nc.sync.dma_start(
                            out=cw_t,
                            in_=cw_d[(kc * 2 + kt) * 128:
                                     (kc * 2 + kt + 1) * 128, :],
                        )
                        nc.vector.tensor_add(g_t, g_t, cw_t)
                    g3 = g_t.rearrange("p (o s) -> p o s", s=6)

                    m = msk.tile([128, O], f32, tag="m")
                    nc.vector.tensor_reduce(m, g3, axis=mybir.AxisListType.X,
                                            op=Alu.max)

                    # one-hot planes e[s][o] (multi-hot only on exact ties)
                    e_t = msk.tile([128, 6, O], bf16, tag="e")
                    gsb = g_t.rearrange("p (o s) -> p s o", s=6)
                    m_b = m.unsqueeze(1).broadcast_to([128, 6, O])
                    nc.vector.tensor_tensor(e_t, gsb, m_b, op=Alu.is_ge)

                    # cols storage order: [col2, col1, col3, col0]
                    # col0=e3+e4+e5  col1=e1+e2+e5  col2=e0+e2+e4  col3=e0+e1+e3
                    ev = e_t  # [128, 6, 512]
                    t2 = msk.tile([128, 2, O], bf16, tag="t2")
                    # t2 = [e0+e2, e1+e2]
                    eng_a.tensor_add(t2, ev[:, 0:2, :],
                                     ev[:, 2:3, :].broadcast_to([128, 2, O]))
                    cols = msk.tile([128, 4, O], bf16, tag="cols")
                    # [col2, col1] = t2 + [e4, e5]
                    eng_a.tensor_add(cols[:, 0:2, :], t2, ev[:, 4:6, :])
                    s2 = msk.tile([128, 2, O], bf16, tag="s2")
                    # s2 = [e0+e1, e4+e5]  (cheap on DVE; Pool's 0.42 eff loses)
                    nc.vector.tensor_add(s2, ev[:, 0::4, :], ev[:, 1::4, :])
                    # [col3, col0] = s2 + e3
                    eng_a.tensor_add(cols[:, 2:4, :], s2,
                                     ev[:, 3:4, :].broadcast_to([128, 2, O]))

                    # wmT tiles j = kt*4 + i, directly in SBUF:
                    # wmt[kc][p, j, o] = wT[p, i, o] * col_i[p, o]
                    eng_m.tensor_mul(
                        wmt[kc][:, kt * 4:(kt + 1) * 4, :].rearrange(
                            "p a b -> p (a b)"),
                        w_tiles[kt],
                        cols.rearrange("p s b -> p (s b)"),
                    )

                # xt prefetch at chunk boundaries
                for s_, h_ in XT_PLAN[kc]:
                    issue_xt(s_, h_)

            # ---------------- phase 2: GEMM ------------------------------
            for st in range(N_STRIP):
                issue_xt(st, 0); issue_xt(st, 1)
                accs = [ps_gemm.tile([128, O], f32, tag="acc", name=f"acc{st}_{i}")
                        for i in range(2)]
                early = st < N_EARLY and _os.environ.get("KV2_OSL", "1") == "1"
                act_out = (st < N_EARLY) or _os.environ.get("KV2_OUT") == "act"
                for h in range(2):
                    if act_out:
                        # bias as the zeroing first matmul; chunk-0 o-sliced
                        # so columns start as each ot's wmT copy lands
                        nc.tensor.matmul(accs[h], ones_t, bias_bf,
                                         start=True, stop=False,
                                         skip_group_check=True)
                    for kc in range(N_KC):
                        for j in range(8):
                            tl = (kc % 2) * 8 + j
                            lhs = xt_tiles[(st, kc // 2)][:, tl,
                                                          h * 128:(h + 1) * 128]
                            if early and kc == 0:
                                for ot in range(N_OT):
                                    osl = slice(ot * 128, (ot + 1) * 128)
                                    nc.tensor.matmul(
                                        accs[h][:, osl], lhs,
                                        wmt[kc][:, j, osl],
                                        start=False, stop=False,
                                        skip_group_check=True,
                                    )
                            else:
                                nc.tensor.matmul(
                                    accs[h], lhs, wmt[kc][:, j, :],
                                    start=(not act_out and kc == 0 and j == 0),
                                    stop=(kc == N_KC - 1 and j == 7),
                                    skip_group_check=act_out,
                                )
                for h in range(2):
                    o_t = out_pool.tile([128, O], bf16, tag="o",
                                        name=f"o_{st}_{h}")
                    if act_out:
                        nc.scalar.copy(o_t, accs[h])
                    else:
                        nc.vector.tensor_add(o_t, accs[h], bias_s)
                    nc.sync.dma_start(
                        out=out_d[st * TS + h * 128:st * TS + (h + 1) * 128, :],
                        in_=o_t,
                    )

            # PE p-state warmers: lowest-priority (program-end) rank-1
            # matmuls into a scratch bank. They run only when PE is idle
            # (the cold phase-1 window), keeping the ramp at full speed for
            # the real GEMM. Count must drain within the early idle.
            n_warm = int(_os.environ.get("KV3_WARM", "110"))
            warm_w = int(_os.environ.get("KV3_WARMW", "512"))
            if n_warm:
                scratch = ps_warm.tile([128, O], f32, name="scratch")
                for wi in range(n_warm):
                    nc.tensor.matmul(scratch[:, 0:warm_w], ones_t,
                                     bias_bf[:, 0:warm_w],
                                     start=True, stop=True,
                                     skip_group_check=True)

    nc.compile()
    return nc


def _get_program(mode, const_c=None):
    key = mode
    if key not in _prog_cache:
        _prog_cache[key] = _build_program(mode)
    return _prog_cache[key]


# s-plane order for cols/w packing: cols tile holds [col2, col1, col3, col0]
PLANE_ORDER = (2, 1, 3, 0)


def _k_index():
    """K_IDX[tile, p] = source k for GEMM tile `tile`=kc*8+j, partition p.
    Tile j = kbt*4 + i: partition p = kb_local (block within the kbt-half of
    chunk kc), fixed k-offset s = PLANE_ORDER[i] per tile."""
    tl = np.arange(32)[:, None]          # kc*8 + j
    p = np.arange(128)[None, :]
    kc = tl // 8
    j = tl % 8
    kbt = j // 4
    i = j % 4
    s = np.asarray(PLANE_ORDER)[i]
    return 4 * (kc * KC_B + kbt * 128 + p) + s


def _pack_host(x, weight, bias, gumbel_noise):
    import ml_dtypes
    bf16 = ml_dtypes.bfloat16

    x2 = np.asarray(x, np.float32).reshape(T, K).astype(bf16)
    kidx = _k_index().reshape(-1)
    # xt_packed[strip, p, tile, t] = x[strip*TS + t, K_IDX[tile, p]]
    xg = x2[:, kidx].reshape(N_STRIP, TS, 32, 128)
    xt_packed = np.ascontiguousarray(xg.transpose(0, 3, 2, 1)).reshape(
        N_STRIP * 128, 32 * TS
    )

    w = np.asarray(weight, np.float32).astype(bf16)
    b = np.asarray(bias, np.float32).reshape(1, O_FULL)
    g = np.asarray(gumbel_noise, np.float32).reshape(O_FULL, GUM_COLS)
    return xt_packed, w, b, g


def _pack_w_core(w_core):
    """wT_packed[(kc,kbt)*128 + p, i*512 + o] = w_core[o, K_IDX[kc*8+kbt*4+i, p]]"""
    wT = np.ascontiguousarray(w_core.T)              # [K, O] bf16
    kidx = _k_index().reshape(N_KC, 2, 4, 128)       # [kc, kbt, i, p]
    rows = wT[kidx]                                  # [kc, kbt, i, p, O]
    return np.ascontiguousarray(
        rows.transpose(0, 1, 3, 2, 4)).reshape(8 * 128, 4 * O)


def _pack_g_core(g_core):
    """gT_packed[kb, o*6 + s] from g_core[o, kb*6 + s]."""
    g3 = g_core.reshape(O, B, 6)
    return np.ascontiguousarray(g3.transpose(1, 0, 2)).reshape(B, O * 6)


def kernel(x, weight, bias, choice_weights, gumbel_noise):
    from concourse.bass_utils import run_bass_kernel_spmd

    cw = np.asarray(choice_weights, np.float32)
    c0 = float(cw.flat[0])
    is_const = bool((cw == c0).all())
    mode = "const" if is_const else "full"
    nc = _get_program(mode)

    xt_packed, w, b, g = _pack_host(x, weight, bias, gumbel_noise)

    in_maps = []
    for c in range(N_CORES):
        rows = slice(c * O, (c + 1) * O)
        m = {
            "xt": xt_packed,
            "w": _pack_w_core(w[rows]),
            "b": np.ascontiguousarray(b[:, rows]),
            "g": _pack_g_core(g[rows]),
        }
        if mode == "full":
            m["cw"] = _pack_g_core(cw.reshape(O_FULL, GUM_COLS)[rows])
        in_maps.append(m)

    res = run_bass_kernel_spmd(nc, in_maps, list(range(N_CORES)))
    parts = [np.asarray(res.results[c]["out"]).astype(np.float32)
             for c in range(N_CORES)]
    out = np.concatenate(parts, axis=1)  # [T, O_FULL]
    return out.reshape(2, 2048, O_FULL)
